# revision 4
# baseline (speedup 1.0000x reference)
"""AttentionBlock (GroupNorm + single-head attention over HW tokens + proj +
residual) as a Bass/Tile kernel for 8 Trainium2 NeuronCores.

Sharding: data-parallel over batch B=32 -> 4 samples per core; 1x1-conv
weights replicated.

Per-sample dataflow on one core (C=256, HW=1024, fp32 in / fp32 out,
float32r matmuls):
  x (2x (128,1024) SBUF tiles, channel-major)
  GroupNorm: per-channel sum (DVE reduce) + sumsq (DVE tensor_tensor_reduce);
    group (8) sums via tiny fp32 matmul against a 0/1 group mask;
    rstd = exp(-0.5*ln(var+eps)) on ACT (keeps ACT on the
    natural_log_exp_and_others table set all kernel long);
    per-channel scale/shift broadcast back via tiny matmul against maskT;
    h = x*sc + sh in one DVE tensor_scalar pass (written as float32r).
  QKV: q,k in (C, HW) layout (lhsT = w^T chunks); vT in (HW, C) layout
    (lhsT = h chunks). 1/sqrt(C) folded into w_q/b_q on the host.
  Scores transposed: sT[j,i] = sum_c k[c,j] q[c,i]; softmax over j
    (= partitions) without max-subtraction (scores are O(6), exp is safe in
    fp32): pT = exp(sT) on ACT; denominators via ones-column matmul.
  attnout[c,i] = sum_j vT[j,c] pT[j,i] (lhsT = vT chunks), v-bias folded in
    as per-partition add on the PSUM->SBUF copy.
  proj: lhsT = w_proj^T chunks over attnout; final out =
    (proj*recip(denom) + b_proj) + x with recip broadcast across partitions
    by GpSimd.
"""

import numpy as np

import concourse.bacc as bacc
import concourse.tile as tile
import concourse.mybir as mybir
from concourse.bass_utils import run_bass_kernel_spmd

F32 = mybir.dt.float32
F32R = mybir.dt.float32r
ALU = mybir.AluOpType
ACTF = mybir.ActivationFunctionType

N_CORES = 8
B, C, H, W = 32, 256, 32, 32
HW = H * W          # 1024
S = B // N_CORES    # 4 samples per core
G = 8               # groups
CG = C // G         # 32 channels per group
EPS = 1e-5
NC2 = C // 128      # channel chunks of 128
NH2 = HW // 512     # hw halves of 512


def _emit_sample(nc, pools, wt, s, x_ap, out_ap):
    """Emit one sample's instruction stream."""
    sb, ps = pools

    inv_n = 1.0 / float(CG * HW)

    # ---- load x + per-channel stats -------------------------------------
    xt = []
    st = []
    for ci in range(NC2):
        x_t = sb.tile([128, HW], F32, name=f"x_s{s}c{ci}", tag=f"x{ci}", bufs=2)
        nc.sync.dma_start(x_t[:], x_ap[s, ci * 128:(ci + 1) * 128, :])
        xt.append(x_t)
        st_t = sb.tile([128, 2], F32, name=f"st_s{s}c{ci}", tag=f"st{ci}", bufs=2)
        nc.vector.reduce_sum(st_t[:, 0:1], x_t[:], axis=mybir.AxisListType.X)
        scr = sb.tile([128, HW], F32, name=f"scr_s{s}c{ci}", tag="scr", bufs=2)
        nc.scalar.activation(scr[:], x_t[:], ACTF.Square,
                             accum_out=st_t[:, 1:2])
        st.append(st_t)

    # ---- group stats (8 groups) ----------------------------------------
    gst = ps.tile([8, 2], F32, name=f"gst_s{s}", tag="small", bufs=2)
    for ci in range(NC2):
        nc.tensor.matmul(gst[:], wt["gmask"][:, ci * G:(ci + 1) * G], st[ci][:],
                         start=(ci == 0), stop=(ci == NC2 - 1))
    gv = sb.tile([8, 2], F32, name=f"gv_s{s}", tag="gv", bufs=2)
    # gv[:,0] = mean, gv[:,1] = rstd
    nc.scalar.mul(gv[:, 0:1], gst[:, 0:1], inv_n)
    ex2 = sb.tile([8, 1], F32, name=f"ex2_s{s}", tag="ex2", bufs=2)
    nc.scalar.mul(ex2[:], gst[:, 1:2], inv_n)
    msq = sb.tile([8, 1], F32, name=f"msq_s{s}", tag="msq", bufs=2)
    nc.vector.tensor_mul(msq[:], gv[:, 0:1], gv[:, 0:1])
    var = sb.tile([8, 1], F32, name=f"var_s{s}", tag="var", bufs=2)
    # var = (ex2 + eps) - mean^2
    nc.vector.scalar_tensor_tensor(var[:], in0=ex2[:], scalar=EPS, in1=msq[:],
                                   op0=ALU.add, op1=ALU.subtract)
    lnv = sb.tile([8, 1], F32, name=f"lnv_s{s}", tag="lnv", bufs=2)
    nc.scalar.activation(lnv[:], var[:], ACTF.Ln)
    nc.scalar.activation(gv[:, 1:2], lnv[:], ACTF.Exp, scale=-0.5)

    # ---- normalize: h = x*sc + sh --------------------------------------
    ht = []
    for ci in range(NC2):
        mr = ps.tile([128, 2], F32, name=f"mr_s{s}c{ci}", tag="small", bufs=2)
        nc.tensor.matmul(mr[:], wt["maskT"][:, ci * 128:(ci + 1) * 128], gv[:],
                         start=True, stop=True)
        sc = sb.tile([128, 1], F32, name=f"sc_s{s}c{ci}", tag="sc", bufs=3)
        nc.vector.tensor_mul(sc[:], wt["gamma"][ci][:], mr[:, 1:2])
        tmp = sb.tile([128, 1], F32, name=f"tmp_s{s}c{ci}", tag="tmp", bufs=3)
        nc.vector.tensor_mul(tmp[:], mr[:, 0:1], sc[:])
        sh = sb.tile([128, 1], F32, name=f"sh_s{s}c{ci}", tag="sh", bufs=3)
        nc.vector.tensor_sub(sh[:], wt["beta"][ci][:], tmp[:])
        h_t = sb.tile([128, HW], F32R, name=f"h_s{s}c{ci}", tag=f"h{ci}", bufs=2)
        nc.vector.tensor_scalar(h_t[:], xt[ci][:], sc[:], sh[:],
                                op0=ALU.mult, op1=ALU.add)
        ht.append(h_t)

    # ---- QKV projections ------------------------------------------------
    # q, k in (C, HW) layout
    q_sb, k_sb = [], []
    for ci in range(NC2):
        q_t = sb.tile([128, HW], F32R, name=f"q_s{s}c{ci}", tag=f"q{ci}", bufs=2)
        k_t = sb.tile([128, HW], F32R, name=f"k_s{s}c{ci}", tag=f"k{ci}", bufs=2)
        for ih in range(NH2):
            hs = slice(ih * 512, (ih + 1) * 512)
            qp = ps.tile([128, 512], F32, name=f"qp_s{s}c{ci}h{ih}", tag="big", bufs=4)
            for cc in range(NC2):
                nc.tensor.matmul(
                    qp[:],
                    wt["wq"][cc][:, ci * 128:(ci + 1) * 128],
                    ht[cc][:, hs],
                    start=(cc == 0), stop=(cc == NC2 - 1))
            nc.scalar.add(q_t[:, hs], qp[:], wt["bq"][ci][:])
            kp = ps.tile([128, 512], F32, name=f"kp_s{s}c{ci}h{ih}", tag="big", bufs=4)
            for cc in range(NC2):
                nc.tensor.matmul(
                    kp[:],
                    wt["wq"][cc][:, C + ci * 128:C + (ci + 1) * 128],
                    ht[cc][:, hs],
                    start=(cc == 0), stop=(cc == NC2 - 1))
            nc.scalar.add(k_t[:, hs], kp[:], wt["bk"][ci][:])
        q_sb.append(q_t)
        k_sb.append(k_t)

    # vT in (HW, C) layout, 8 chunks of 128 positions
    v_sb = []
    for j in range(HW // 128):
        vp = ps.tile([128, C], F32, name=f"vp_s{s}j{j}", tag="big", bufs=4)
        for cc in range(NC2):
            nc.tensor.matmul(
                vp[:],
                ht[cc][:, j * 128:(j + 1) * 128],
                wt["wq"][cc][:, 2 * C:3 * C],
                start=(cc == 0), stop=(cc == NC2 - 1))
        v_t = sb.tile([128, C], F32R, name=f"v_s{s}j{j}", tag="vt", bufs=12)
        nc.vector.tensor_copy(v_t[:], vp[:])
        v_sb.append(v_t)

    # ---- attention ------------------------------------------------------
    ao_sb = [[None] * NC2 for _ in range(NH2)]
    rb = [None] * NH2
    for ih in range(NH2):
        hs = slice(ih * 512, (ih + 1) * 512)
        # scores^T + exp, 8 chunks of 128 key positions
        pt = []
        for j in range(HW // 128):
            sp = ps.tile([128, 512], F32, name=f"sp_s{s}h{ih}j{j}", tag="big", bufs=4)
            for cc in range(NC2):
                nc.tensor.matmul(
                    sp[:],
                    k_sb[cc][:, j * 128:(j + 1) * 128],
                    q_sb[cc][:, hs],
                    start=(cc == 0), stop=(cc == NC2 - 1))
            p_t = sb.tile([128, 512], F32R, name=f"p_s{s}h{ih}j{j}", tag="pt", bufs=10)
            nc.scalar.activation(p_t[:], sp[:], ACTF.Exp)
            pt.append(p_t)
        # attnout[c, i] accumulation over j
        for ci in range(NC2):
            ao = ps.tile([128, 512], F32, name=f"ao_s{s}h{ih}c{ci}", tag="ao", bufs=2)
            for j in range(HW // 128):
                nc.tensor.matmul(ao[:], v_sb[j][:, ci * 128:(ci + 1) * 128], pt[j][:],
                                 start=(j == 0), stop=(j == HW // 128 - 1))
            ao_t = sb.tile([128, 512], F32R, name=f"aot_s{s}h{ih}c{ci}", tag="ao_sb",
                           bufs=6)
            nc.vector.tensor_scalar_add(ao_t[:], ao[:], wt["bv"][ci][:])
            ao_sb[ih][ci] = ao_t
        # softmax denominators (sum over j = partitions) via ones matmul
        dn = ps.tile([1, 512], F32, name=f"dn_s{s}h{ih}", tag="small", bufs=2)
        for j in range(HW // 128):
            nc.tensor.matmul(dn[:], wt["ones_col"][:], pt[j][:],
                             start=(j == 0), stop=(j == HW // 128 - 1))
        dn_sb = sb.tile([1, 512], F32R, name=f"dnsb_s{s}h{ih}", tag="dnsb", bufs=2)
        nc.vector.tensor_copy(dn_sb[:], dn[:])
        rbp = ps.tile([128, 512], F32, name=f"rbp_s{s}h{ih}", tag="big", bufs=4)
        nc.tensor.matmul(rbp[:], wt["ones_row"][:], dn_sb[:], start=True, stop=True)
        rb_t = sb.tile([128, 512], F32, name=f"rb_s{s}h{ih}", tag="rb", bufs=3)
        nc.vector.reciprocal(rb_t[:], rbp[:])
        rb[ih] = rb_t

    # ---- projection + epilogue -----------------------------------------
    for ci in range(NC2):
        for ih in range(NH2):
            hs = slice(ih * 512, (ih + 1) * 512)
            pp = ps.tile([128, 512], F32, name=f"pp_s{s}c{ci}h{ih}", tag="ao", bufs=2)
            for cc in range(NC2):
                nc.tensor.matmul(
                    pp[:],
                    wt["wp"][cc][:, ci * 128:(ci + 1) * 128],
                    ao_sb[ih][cc][:],
                    start=(cc == 0), stop=(cc == NC2 - 1))
            t_t = sb.tile([128, 512], F32, name=f"t_s{s}c{ci}h{ih}", tag="t", bufs=3)
            nc.vector.tensor_mul(t_t[:], pp[:], rb[ih][:])
            o_t = sb.tile([128, 512], F32, name=f"o_s{s}c{ci}h{ih}", tag="o", bufs=4)
            nc.vector.scalar_tensor_tensor(
                o_t[:], in0=t_t[:], scalar=wt["bp"][ci][:], in1=xt[ci][:, hs],
                op0=ALU.add, op1=ALU.add)
            nc.sync.dma_start(out_ap[s, ci * 128:(ci + 1) * 128, hs], o_t[:])


def build_program(reps=1):
    nc = bacc.Bacc("TRN2", target_bir_lowering=False, debug=False,
                   enable_asserts=False, num_devices=N_CORES)

    x_ap = nc.dram_tensor("x", [S, C, HW], F32, kind="ExternalInput").ap()
    wq_ap = nc.dram_tensor("wqkvT", [C, 3 * C], F32R, kind="ExternalInput").ap()
    wp_ap = nc.dram_tensor("wprojT", [C, C], F32R, kind="ExternalInput").ap()
    bq_ap = nc.dram_tensor("bq", [NC2, 128, 1], F32, kind="ExternalInput").ap()
    bk_ap = nc.dram_tensor("bk", [NC2, 128, 1], F32, kind="ExternalInput").ap()
    bv_ap = nc.dram_tensor("bv", [NC2, 128, 1], F32, kind="ExternalInput").ap()
    bp_ap = nc.dram_tensor("bp", [NC2, 128, 1], F32, kind="ExternalInput").ap()
    gam_ap = nc.dram_tensor("gamma", [NC2, 128, 1], F32, kind="ExternalInput").ap()
    bet_ap = nc.dram_tensor("beta", [NC2, 128, 1], F32, kind="ExternalInput").ap()
    gm_ap = nc.dram_tensor("gmask", [128, NC2 * G], F32, kind="ExternalInput").ap()
    gmt_ap = nc.dram_tensor("gmaskT", [G, C], F32, kind="ExternalInput").ap()
    ones_ap = nc.dram_tensor("ones", [128, 1], F32R, kind="ExternalInput").ap()
    onesr_ap = nc.dram_tensor("ones_row", [1, 128], F32R, kind="ExternalInput").ap()
    out_ap = nc.dram_tensor("out", [S, C, HW], F32, kind="ExternalOutput").ap()

    with tile.TileContext(nc) as tc:
        with (
            tc.tile_pool(name="wpool", bufs=1) as wp,
            tc.tile_pool(name="sb", bufs=2) as sb,
            tc.tile_pool(name="ps", bufs=2, space="PSUM") as ps,
        ):
            # persistent weights / constants
            wq0 = wp.tile([128, 3 * C], F32R, name="wq0", tag="wq0")
            nc.sync.dma_start(wq0[:], wq_ap[0:128, :])
            wq1 = wp.tile([128, 3 * C], F32R, name="wq1", tag="wq1")
            nc.sync.dma_start(wq1[:], wq_ap[128:256, :])
            wp0 = wp.tile([128, C], F32R, name="wp0", tag="wp0")
            nc.sync.dma_start(wp0[:], wp_ap[0:128, :])
            wp1 = wp.tile([128, C], F32R, name="wp1", tag="wp1")
            nc.sync.dma_start(wp1[:], wp_ap[128:256, :])
            gmask = wp.tile([128, NC2 * G], F32, name="gmask", tag="gmask")
            nc.sync.dma_start(gmask[:], gm_ap[:])
            maskT = wp.tile([G, C], F32, name="maskT", tag="maskT")
            nc.sync.dma_start(maskT[:], gmt_ap[:])
            ones_col = wp.tile([128, 1], F32R, name="ones_col", tag="ones_col")
            nc.sync.dma_start(ones_col[:], ones_ap[:])
            ones_row = wp.tile([1, 128], F32R, name="ones_row", tag="ones_row")
            nc.sync.dma_start(ones_row[:], onesr_ap[:])

            def _load_cols(name, ap):
                ts = []
                for ci in range(NC2):
                    t = wp.tile([128, 1], F32, name=f"{name}{ci}", tag=f"{name}{ci}")
                    nc.sync.dma_start(t[:], ap[ci])
                    ts.append(t)
                return ts

            wt = {
                "wq": [wq0, wq1],
                "wp": [wp0, wp1],
                "gmask": gmask,
                "maskT": maskT,
                "ones_col": ones_col,
                "ones_row": ones_row,
                "bq": _load_cols("bq", bq_ap),
                "bk": _load_cols("bk", bk_ap),
                "bv": _load_cols("bv", bv_ap),
                "bp": _load_cols("bp", bp_ap),
                "gamma": _load_cols("gam", gam_ap),
                "beta": _load_cols("bet", bet_ap),
            }

            for _ in range(reps):
                for s in range(S):
                    _emit_sample(nc, (sb, ps), wt, s, x_ap, out_ap)

    nc.compile()
    return nc


def prep_inputs(x, gamma, beta, w_qkv, b_qkv, w_proj, b_proj):
    """Host-side prep: shard x over cores, transpose/scale weights."""
    x = np.ascontiguousarray(x, dtype=np.float32).reshape(B, C, HW)
    x_shards = x.reshape(N_CORES, S, C, HW)

    scale = np.float32(1.0 / np.sqrt(np.float32(C)))
    wqkvT = np.ascontiguousarray(np.asarray(w_qkv, np.float32).T)  # (C, 3C)
    wqkvT[:, 0:C] *= scale
    b_qkv = np.asarray(b_qkv, np.float32).copy()
    bq = (b_qkv[0:C] * scale).reshape(NC2, 128, 1)
    bk = b_qkv[C:2 * C].reshape(NC2, 128, 1)
    bv = b_qkv[2 * C:3 * C].reshape(NC2, 128, 1)
    wprojT = np.ascontiguousarray(np.asarray(w_proj, np.float32).T)
    bp = np.asarray(b_proj, np.float32).reshape(NC2, 128, 1)
    gam = np.asarray(gamma, np.float32).reshape(NC2, 128, 1)
    bet = np.asarray(beta, np.float32).reshape(NC2, 128, 1)

    gmask = np.zeros((128, NC2 * G), np.float32)
    gmaskT = np.zeros((G, C), np.float32)
    for c in range(C):
        g = c // CG
        gmaskT[g, c] = 1.0
        gmask[c % 128, (c // 128) * G + g] = 1.0

    shared = {
        "wqkvT": np.ascontiguousarray(wqkvT),
        "wprojT": wprojT,
        "bq": np.ascontiguousarray(bq), "bk": np.ascontiguousarray(bk),
        "bv": np.ascontiguousarray(bv), "bp": np.ascontiguousarray(bp),
        "gamma": np.ascontiguousarray(gam), "beta": np.ascontiguousarray(bet),
        "gmask": gmask, "gmaskT": gmaskT,
        "ones": np.ones((128, 1), np.float32),
        "ones_row": np.ones((1, 128), np.float32),
    }
    return [dict(shared, x=np.ascontiguousarray(x_shards[i]))
            for i in range(N_CORES)]


_NC_CACHE = {}


def kernel(x, gamma, beta, w_qkv, b_qkv, w_proj, b_proj):
    if "nc" not in _NC_CACHE:
        _NC_CACHE["nc"] = build_program()
    nc = _NC_CACHE["nc"]
    in_maps = prep_inputs(x, gamma, beta, w_qkv, b_qkv, w_proj, b_proj)
    res = run_bass_kernel_spmd(nc, in_maps, list(range(N_CORES)))
    out = np.stack([res.results[i]["out"] for i in range(N_CORES)])
    return out.reshape(B, C, H, W)


# revision 17
# speedup vs baseline: 240.8007x; 240.8007x over previous
"""AttentionBlock (GroupNorm + single-head attention over HW tokens + proj +
residual) as a Bass/Tile kernel for 8 Trainium2 NeuronCores.

Sharding: data-parallel over batch B=32 -> 4 samples per core; 1x1-conv
weights replicated.

Per-sample dataflow on one core (C=256, HW=1024, fp32 in / fp32 out,
float32r matmuls):
  x (2x (128,1024) SBUF tiles, channel-major)
  GroupNorm: per-channel sum (DVE reduce) + sumsq (DVE tensor_tensor_reduce);
    group (8) sums via tiny fp32 matmul against a 0/1 group mask;
    rstd = exp(-0.5*ln(var+eps)) on ACT (keeps ACT on the
    natural_log_exp_and_others table set all kernel long);
    per-channel scale/shift broadcast back via tiny matmul against maskT;
    h = x*sc + sh in one DVE tensor_scalar pass (written as float32r).
  QKV: q,k in (C, HW) layout (lhsT = w^T chunks); vT in (HW, C) layout
    (lhsT = h chunks). 1/sqrt(C) folded into w_q/b_q on the host.
  Scores transposed: sT[j,i] = sum_c k[c,j] q[c,i]; softmax over j
    (= partitions) without max-subtraction (scores are O(6), exp is safe in
    fp32): pT = exp(sT) on ACT; denominators via ones-column matmul.
  attnout[c,i] = sum_j vT[j,c] pT[j,i] (lhsT = vT chunks), v-bias folded in
    as per-partition add on the PSUM->SBUF copy.
  proj: lhsT = w_proj^T chunks over attnout; final out =
    (proj*recip(denom) + b_proj) + x with recip broadcast across partitions
    by GpSimd.
"""

import numpy as np

import concourse.bacc as bacc
import concourse.tile as tile
import concourse.mybir as mybir
from concourse.bass_utils import run_bass_kernel_spmd

F32 = mybir.dt.float32
F32R = mybir.dt.float32r
ALU = mybir.AluOpType
ACTF = mybir.ActivationFunctionType

N_CORES = 8
B, C, H, W = 32, 256, 32, 32
HW = H * W          # 1024
S = B // N_CORES    # 4 samples per core
G = 8               # groups
CG = C // G         # 32 channels per group
EPS = 1e-5
NC2 = C // 128      # channel chunks of 128
NH2 = HW // 512     # hw halves of 512


def _emit_stats(nc, pools, wt, s, x_ap):
    """Load x and run the whole GroupNorm scalar pipeline down to per-channel
    scale/shift columns. Emitted in the kernel prologue for every sample so
    the steady-state engine streams carry no stat dependencies. Stats run on
    ACT (Copy/Square + accum, same table set as exp); inv_n is folded into
    the group-sum mask, gamma into the broadcast mask, rsqrt is a DVE
    Newton iteration."""
    sb, ps = pools
    I32 = mybir.dt.int32

    xt, st = [], []
    for ci in range(NC2):
        x_t = sb.tile([128, HW], F32, name=f"x_s{s}c{ci}", tag=f"x{ci}", bufs=S)
        nc.sync.dma_start(x_t[:], x_ap[s, ci * 128:(ci + 1) * 128, :])
        xt.append(x_t)
        st_t = sb.tile([128, 2], F32, name=f"st_s{s}c{ci}", tag=f"st{ci}", bufs=S)
        scr = sb.tile([128, HW], F32, name=f"scr_s{s}c{ci}", tag="scr", bufs=2)
        nc.scalar.activation(scr[:], x_t[:], ACTF.Copy, accum_out=st_t[:, 0:1])
        scr2 = sb.tile([128, HW], F32, name=f"sq_s{s}c{ci}", tag="scr", bufs=2)
        nc.scalar.activation(scr2[:], x_t[:], ACTF.Square, accum_out=st_t[:, 1:2])
        st.append(st_t)

    # group stats: gst = [mean, ex2] (gmask carries 1/n)
    gst = ps.tile([8, 2], F32, name=f"gst_s{s}", tag="small", bufs=1)
    for ci in range(NC2):
        nc.tensor.matmul(gst[:], wt["gmask"][:, ci * G:(ci + 1) * G], st[ci][:],
                         start=(ci == 0), stop=(ci == NC2 - 1))
    gsb = sb.tile([8, 2], F32, name=f"gsb_s{s}", tag="gsb", bufs=2)
    nc.vector.tensor_copy(gsb[:], gst[:])
    msq = sb.tile([8, 1], F32, name=f"msq_s{s}", tag="msq", bufs=2)
    nc.vector.tensor_mul(msq[:], gsb[:, 0:1], gsb[:, 0:1])
    var = sb.tile([8, 1], F32, name=f"var_s{s}", tag="var", bufs=2)
    nc.vector.scalar_tensor_tensor(var[:], in0=gsb[:, 1:2], scalar=EPS,
                                   in1=msq[:], op0=ALU.add, op1=ALU.subtract)
    # rstd = rsqrt(var): fast-inverse-sqrt bit trick + 2 Newton steps
    ish = sb.tile([8, 1], I32, name=f"ish_s{s}", tag="ish", bufs=2)
    nc.vector.tensor_scalar(ish[:], var[:].bitcast(I32), 1, None,
                            op0=ALU.arith_shift_right)
    yib = sb.tile([8, 1], I32, name=f"yib_s{s}", tag="yib", bufs=2)
    nc.vector.tensor_tensor(yib[:], wt["magic"][0:8, :].bitcast(I32), ish[:],
                            op=ALU.subtract)
    y = yib[:].bitcast(F32)
    for it in range(2):
        ta = sb.tile([8, 1], F32, name=f"ta{it}_s{s}", tag=f"ta{it}", bufs=2)
        nc.vector.tensor_mul(ta[:], y, y)
        tb = sb.tile([8, 1], F32, name=f"tb{it}_s{s}", tag=f"tb{it}", bufs=2)
        nc.vector.tensor_mul(tb[:], ta[:], var[:])
        tcr = sb.tile([8, 1], F32, name=f"tc{it}_s{s}", tag=f"tc{it}", bufs=2)
        nc.vector.tensor_scalar(tcr[:], tb[:], -0.5, 1.5, op0=ALU.mult,
                                op1=ALU.add)
        yn = sb.tile([8, 1], F32, name=f"yn{it}_s{s}", tag=f"yn{it}", bufs=2)
        nc.vector.tensor_mul(yn[:], y, tcr[:])
        y = yn[:]
    # gv2 = [rstd, mean*rstd] feeds the gamma-scaled broadcast matmul
    gv2 = sb.tile([8, 2], F32, name=f"gv2_s{s}", tag="gv2", bufs=2)
    nc.vector.tensor_copy(gv2[:, 0:1], y)
    nc.vector.tensor_mul(gv2[:, 1:2], y, gsb[:, 0:1])

    scc, shc = [], []
    for ci in range(NC2):
        # mr = [gamma*rstd, gamma*mean*rstd] per channel
        mr = ps.tile([128, 2], F32, name=f"mr_s{s}c{ci}", tag="small", bufs=1)
        nc.tensor.matmul(mr[:], wt["maskTg"][:, ci * 128:(ci + 1) * 128],
                         gv2[:], start=True, stop=True)
        sc_t = sb.tile([128, 1], F32, name=f"scc_s{s}c{ci}", tag=f"scc{ci}",
                       bufs=S)
        nc.vector.tensor_copy(sc_t[:], mr[:, 0:1])
        sh_t = sb.tile([128, 1], F32, name=f"sh_s{s}c{ci}", tag=f"sh{ci}",
                       bufs=S)
        nc.vector.tensor_sub(sh_t[:], wt["beta"][ci], mr[:, 1:2])
        scc.append(sc_t)
        shc.append(sh_t)
    return xt, scc, shc


def _emit_h(nc, pools, wt, s, stats):
    """Apply normalization: h = x*scc + sh (float32r), 2 DVE passes."""
    sb, ps = pools
    xt, scc, shc = stats
    ht = []
    for ci in range(NC2):
        h_t = sb.tile([128, HW], F32R, name=f"h_s{s}c{ci}", tag=f"h{ci}", bufs=2)
        nc.vector.tensor_scalar(h_t[:], xt[ci][:], scc[ci][:], shc[ci][:],
                                op0=ALU.mult, op1=ALU.add)
        ht.append(h_t)
    return xt, ht


def _emit_attn(nc, pools, wt, s, xt, ht, out_ap, mid_cb=None):
    """QKV + attention + projection + residual for sample s. mid_cb is
    emitted between the two query-half blocks (used to interleave the next
    sample's GroupNorm at a priority below this sample's first half)."""
    sb, ps = pools

    # q, k in (C, HW) layout
    q_sb, k_sb = [], []
    for ci in range(NC2):
        q_t = sb.tile([128, HW], F32R, name=f"q_s{s}c{ci}", tag=f"q{ci}", bufs=3)
        k_t = sb.tile([128, HW], F32R, name=f"k_s{s}c{ci}", tag=f"k{ci}", bufs=3)
        for ih in range(NH2):
            hs = slice(ih * 512, (ih + 1) * 512)
            qp = ps.tile([128, 512], F32, name=f"qp_s{s}c{ci}h{ih}", tag="big", bufs=4)
            for cc in range(NC2):
                nc.tensor.matmul(
                    qp[:],
                    wt["wq"][cc][:, ci * 128:(ci + 1) * 128],
                    ht[cc][:, hs],
                    start=(cc == 0), stop=(cc == NC2 - 1))
            nc.scalar.add(q_t[:, hs], qp[:], wt["bq"][ci])
            kp = ps.tile([128, 512], F32, name=f"kp_s{s}c{ci}h{ih}", tag="big", bufs=4)
            for cc in range(NC2):
                nc.tensor.matmul(
                    kp[:],
                    wt["wq"][cc][:, C + ci * 128:C + (ci + 1) * 128],
                    ht[cc][:, hs],
                    start=(cc == 0), stop=(cc == NC2 - 1))
            nc.scalar.add(k_t[:, hs], kp[:], wt["bk"][ci])
        q_sb.append(q_t)
        k_sb.append(k_t)

    # vT in (HW, C) layout, 8 chunks of 128 positions
    v_sb = []
    for j in range(HW // 128):
        vp = ps.tile([128, C], F32, name=f"vp_s{s}j{j}", tag="big", bufs=4)
        for cc in range(NC2):
            nc.tensor.matmul(
                vp[:],
                ht[cc][:, j * 128:(j + 1) * 128],
                wt["wq"][cc][:, 2 * C:3 * C],
                start=(cc == 0), stop=(cc == NC2 - 1))
        v_t = sb.tile([128, C], F32R, name=f"v_s{s}j{j}", tag="vt", bufs=12)
        nc.vector.tensor_copy(v_t[:], vp[:])
        v_sb.append(v_t)

    # attention
    mid_res = None
    for ih in range(NH2):
        hs = slice(ih * 512, (ih + 1) * 512)
        pt = []
        for j in range(HW // 128):
            sp = ps.tile([128, 512], F32, name=f"sp_s{s}h{ih}j{j}", tag="big", bufs=4)
            for cc in range(NC2):
                nc.tensor.matmul(
                    sp[:],
                    k_sb[cc][:, j * 128:(j + 1) * 128],
                    q_sb[cc][:, hs],
                    start=(cc == 0), stop=(cc == NC2 - 1))
            p_t = sb.tile([128, 512], F32R, name=f"p_s{s}h{ih}j{j}", tag="pt", bufs=10)
            nc.scalar.activation(p_t[:], sp[:], ACTF.Exp)
            pt.append(p_t)
        # softmax denominators first (short dependent chain: dnsb->rbp->recip)
        dn = ps.tile([1, 512], F32, name=f"dn_s{s}h{ih}", tag="ao", bufs=3)
        for j in range(HW // 128):
            nc.tensor.matmul(dn[:], wt["ones_col"][:], pt[j][:],
                             start=(j == 0), stop=(j == HW // 128 - 1))
        # attnout accumulation hides the denominator chain
        ao_sb = []
        for ci in range(NC2):
            ao = ps.tile([128, 512], F32, name=f"ao_s{s}h{ih}c{ci}", tag="ao", bufs=3)
            for j in range(HW // 128):
                nc.tensor.matmul(ao[:], v_sb[j][:, ci * 128:(ci + 1) * 128],
                                 pt[j][:],
                                 start=(j == 0), stop=(j == HW // 128 - 1))
            ao_t = sb.tile([128, 512], F32R, name=f"aot_s{s}h{ih}c{ci}",
                           tag="ao_sb", bufs=6)
            nc.scalar.add(ao_t[:], ao[:], wt["bv"][ci])
            ao_sb.append(ao_t)
        dn_sb = sb.tile([1, 512], F32R, name=f"dnsb_s{s}h{ih}", tag="dnsb", bufs=2)
        nc.vector.tensor_copy(dn_sb[:], dn[:])
        rbp = ps.tile([128, 512], F32, name=f"rbp_s{s}h{ih}", tag="big", bufs=4)
        nc.tensor.matmul(rbp[:], wt["ones_row"][:], dn_sb[:], start=True,
                         stop=True)
        rb_t = sb.tile([128, 512], F32, name=f"rb_s{s}h{ih}", tag="rb", bufs=3)
        nc.vector.reciprocal(rb_t[:], rbp[:])

        # projection + epilogue for this query half
        hs = slice(ih * 512, (ih + 1) * 512)
        for ci in range(NC2):
            pp = ps.tile([128, 512], F32, name=f"pp_s{s}c{ci}h{ih}", tag="ao", bufs=3)
            for cc in range(NC2):
                nc.tensor.matmul(
                    pp[:],
                    wt["wp"][cc][:, ci * 128:(ci + 1) * 128],
                    ao_sb[cc][:],
                    start=(cc == 0), stop=(cc == NC2 - 1))
            t_t = sb.tile([128, 512], F32, name=f"t_s{s}c{ci}h{ih}", tag="t", bufs=3)
            nc.vector.tensor_mul(t_t[:], pp[:], rb_t[:])
            o_t = sb.tile([128, 512], F32, name=f"o_s{s}c{ci}h{ih}", tag="o", bufs=4)
            nc.vector.scalar_tensor_tensor(
                o_t[:], in0=t_t[:], scalar=wt["bp"][ci], in1=xt[ci][:, hs],
                op0=ALU.add, op1=ALU.add)
            nc.sync.dma_start(out_ap[s, ci * 128:(ci + 1) * 128, hs], o_t[:])

        if ih == 0 and mid_cb is not None:
            mid_res = mid_cb()
    return mid_res


def build_program(reps=1):
    nc = bacc.Bacc("TRN2", target_bir_lowering=False, debug=False,
                   enable_asserts=False, num_devices=N_CORES)

    x_ap = nc.dram_tensor("x", [S, C, HW], F32, kind="ExternalInput").ap()
    wq_ap = nc.dram_tensor("wqkvT", [C, 3 * C], F32R, kind="ExternalInput").ap()
    wp_ap = nc.dram_tensor("wprojT", [C, C], F32R, kind="ExternalInput").ap()
    cv_ap = nc.dram_tensor("cvec", [128, 13], F32, kind="ExternalInput").ap()
    gm_ap = nc.dram_tensor("gmask", [128, NC2 * G], F32, kind="ExternalInput").ap()
    gmt_ap = nc.dram_tensor("gmaskTg", [G, C], F32, kind="ExternalInput").ap()
    ones_ap = nc.dram_tensor("ones", [128, 1], F32R, kind="ExternalInput").ap()
    onesr_ap = nc.dram_tensor("ones_row", [1, 128], F32R, kind="ExternalInput").ap()
    out_ap = nc.dram_tensor("out", [S, C, HW], F32, kind="ExternalOutput").ap()

    with tile.TileContext(nc) as tc:
        with (
            tc.tile_pool(name="wpool", bufs=1) as wp,
            tc.tile_pool(name="sb", bufs=2) as sb,
            tc.tile_pool(name="ps", bufs=2, space="PSUM") as ps,
        ):
            # stats-critical constants first (tiny); the rest can land later
            gmask = wp.tile([128, NC2 * G], F32, name="gmask", tag="gmask")
            nc.sync.dma_start(gmask[:], gm_ap[:])
            cvec = wp.tile([128, 13], F32, name="cvec", tag="cvec")
            nc.sync.dma_start(cvec[:], cv_ap[:])
            maskTg = wp.tile([G, C], F32, name="maskTg", tag="maskTg")
            nc.sync.dma_start(maskTg[:], gmt_ap[:])
            ones_col = wp.tile([128, 1], F32R, name="ones_col", tag="ones_col")
            nc.sync.dma_start(ones_col[:], ones_ap[:])
            ones_row = wp.tile([1, 128], F32R, name="ones_row", tag="ones_row")
            nc.sync.dma_start(ones_row[:], onesr_ap[:])

            def _cols(k):
                return [cvec[:, (k * NC2 + ci):(k * NC2 + ci + 1)]
                        for ci in range(NC2)]

            wt = {
                "gmask": gmask,
                "maskTg": maskTg,
                "ones_col": ones_col,
                "ones_row": ones_row,
                "bq": _cols(0),
                "bk": _cols(1),
                "bv": _cols(2),
                "bp": _cols(3),
                "gamma": _cols(4),
                "beta": _cols(5),
                "magic": cvec[:, 12:13],
            }

            stats = [None] * S
            stats[0] = _emit_stats(nc, (sb, ps), wt, 0, x_ap)
            stats[1] = _emit_stats(nc, (sb, ps), wt, 1, x_ap)

            # big weights after sample 0's x/stats DMAs are in flight
            wq0 = wp.tile([128, 3 * C], F32R, name="wq0", tag="wq0")
            nc.sync.dma_start(wq0[:], wq_ap[0:128, :])
            wq1 = wp.tile([128, 3 * C], F32R, name="wq1", tag="wq1")
            nc.sync.dma_start(wq1[:], wq_ap[128:256, :])
            wp0 = wp.tile([128, C], F32R, name="wp0", tag="wp0")
            nc.sync.dma_start(wp0[:], wp_ap[0:128, :])
            wp1 = wp.tile([128, C], F32R, name="wp1", tag="wp1")
            nc.sync.dma_start(wp1[:], wp_ap[128:256, :])
            wt["wq"] = [wq0, wq1]
            wt["wp"] = [wp0, wp1]
            h0 = _emit_h(nc, (sb, ps), wt, 0, stats[0])

            # global sample sequence across reps: stats(i) lands at
            # mid-attention of sample i-2, h(i) at mid-attention of i-1.
            seq = [(rep, s) for rep in range(reps) for s in range(S)]
            n_seq = len(seq)
            stats_ring = {0: stats[0], 1: stats[1]}

            hs_cur = h0
            for i in range(n_seq):
                xt, ht = hs_cur

                def mid_cb(i=i):
                    if i + 2 < n_seq:
                        stats_ring[i + 2] = _emit_stats(
                            nc, (sb, ps), wt, seq[i + 2][1], x_ap)
                    if i + 1 < n_seq:
                        return _emit_h(nc, (sb, ps), wt, seq[i + 1][1],
                                       stats_ring.pop(i + 1))
                    return None

                hs_cur = _emit_attn(nc, (sb, ps), wt, seq[i][1], xt, ht,
                                    out_ap, mid_cb=mid_cb)

    nc.compile()
    return nc


def prep_inputs(x, gamma, beta, w_qkv, b_qkv, w_proj, b_proj):
    """Host-side prep: shard x over cores, transpose/scale weights."""
    x = np.ascontiguousarray(x, dtype=np.float32).reshape(B, C, HW)
    x_shards = x.reshape(N_CORES, S, C, HW)

    scale = np.float32(1.0 / np.sqrt(np.float32(C)))
    wqkvT = np.ascontiguousarray(np.asarray(w_qkv, np.float32).T)  # (C, 3C)
    wqkvT[:, 0:C] *= scale
    b_qkv = np.asarray(b_qkv, np.float32).copy()
    bq = (b_qkv[0:C] * scale).reshape(NC2, 128)
    bk = b_qkv[C:2 * C].reshape(NC2, 128)
    bv = b_qkv[2 * C:3 * C].reshape(NC2, 128)
    wprojT = np.ascontiguousarray(np.asarray(w_proj, np.float32).T)
    bp = np.asarray(b_proj, np.float32).reshape(NC2, 128)
    gam = np.asarray(gamma, np.float32).reshape(NC2, 128)
    bet = np.asarray(beta, np.float32).reshape(NC2, 128)
    cvec = np.zeros((128, 13), np.float32)
    for k, arr in enumerate([bq, bk, bv, bp, gam, bet]):
        for ci in range(NC2):
            cvec[:, k * NC2 + ci] = arr[ci]
    cvec[:, 12] = np.uint32(0x5F3759DF).view(np.float32)

    inv_n = np.float32(1.0 / (CG * HW))
    gam_flat = np.asarray(gamma, np.float32).reshape(C)
    gmask = np.zeros((128, NC2 * G), np.float32)
    gmaskTg = np.zeros((G, C), np.float32)
    for c in range(C):
        g = c // CG
        gmaskTg[g, c] = gam_flat[c]
        gmask[c % 128, (c // 128) * G + g] = inv_n

    shared = {
        "wqkvT": np.ascontiguousarray(wqkvT),
        "wprojT": wprojT,
        "cvec": cvec,
        "gmask": gmask, "gmaskTg": gmaskTg,
        "ones": np.ones((128, 1), np.float32),
        "ones_row": np.ones((1, 128), np.float32),
    }
    return [dict(shared, x=np.ascontiguousarray(x_shards[i]))
            for i in range(N_CORES)]


_NC_CACHE = {}


def kernel(x, gamma, beta, w_qkv, b_qkv, w_proj, b_proj):
    if "nc" not in _NC_CACHE:
        _NC_CACHE["nc"] = build_program()
    nc = _NC_CACHE["nc"]
    in_maps = prep_inputs(x, gamma, beta, w_qkv, b_qkv, w_proj, b_proj)
    res = run_bass_kernel_spmd(nc, in_maps, list(range(N_CORES)))
    out = np.stack([res.results[i]["out"] for i in range(N_CORES)])
    return out.reshape(B, C, H, W)
